# revision 17
# baseline (speedup 1.0000x reference)
"""Trainium2 Bass kernel for the SG-visibility sampling network.

Math notes (exploited structure):
  - U,V are orthogonal to the unit lobe axis l, so dot(sample_dir, l) == cos(r_phi)
    exactly (up to fp eps).  Hence the SG weight w = exp(sharp*(cos_phi-1)) is a
    per-lobe constant and sum_s(vis*w)/(sum_s w + TINY) = scale_l * sum_s vis with
    scale_l = w/(S*w + TINY), precomputed on host.
  - pre-activation of the hidden layer decomposes as
        pre_h[n,l,s,h] = P_n[h] - C_l[h] - ct[n,l,s]*A_l[h] - st[n,l,s]*B_l[h]
    with P_n = p_n @ W1[:3] + b1,  A_l = sp_l*(U_l@Wd),  B_l = sp_l*(V_l@Wd),
    C_l = cp_l*(l_l@Wd),  Wd = root_rot @ W1[3:].
  - hemisphere mask: cos_term = ct*a_nl + st*b_nl + c_nl with
    a = normals@(sp*U)_l, b = normals@(sp*V)_l, c = normals@(cp*l)_l.

Device schedule highlights:
  - 3 packed I/O args (din = rt|normals|points, wpk = all weights, out).
  - theta block tiles are persistent (pc rows written once at setup); raw
    theta rows are loaded pre-duplicated straight from DRAM with a stride-0
    broadcast AP (1 DMA per sub-chunk instead of 2 SBUF copies).
  - stationary/moving matmul operands use f32->f32r bitcasts (no copies).
  - hemisphere mask full-width: products on DVE, the q1+q2 combine rides a
    SWDGE accumulate-DMA (CCE add), compare is one DVE is_gt vs thr=TINY-c.
  - per-half z accumulates in two [64,HF] PSUM groups; (z+30)*msk is stored
    to SBUF per chunk and ALL sigmoids run in a tail pass with bias (b2-30),
    gated behind the last chunk, so the ACT engine's function table swaps
    Sin<->Sigmoid exactly once for the whole kernel instead of ~44 times.
  - DMA issue is spread across the sync/gpsimd/scalar queues; relus split
    ACT/DVE to balance the three compute engines at ~78% each.
"""

import numpy as np

N, L, S, H = 8192, 128, 8, 16
NCORES = 8
NC = N // NCORES          # rays per core
LPC = 16                  # lobes per chunk
CHUNKS = L // LPC
TINY = 1e-6
KROWS = 116               # {7,7,2} stationary rows
SUBS = ((0, 7), (7, 7), (14, 2))

OFF_ABC = L * 128                 # wcst cols [0, 16384)
OFF_SIG = OFF_ABC + 3 * L         # wabc cols [16384, 16768)
OFF_SUM = OFF_SIG + 512           # wsig cols [16768, 17280)
OFF_CB = OFF_SUM + CHUNKS * L     # wsum cols [17280, 18304)
WCOLS = OFF_CB + 8                # cb cols [18304, 18312)
DIN_ROWS = L * S + 3 + 4          # rt rows | normals.T | [points.T; 1]

_PROG = None


def _build_program():
    import concourse.bass as bass
    import concourse.bacc as bacc
    import concourse.mybir as mybir
    import concourse.tile as tile

    f32 = mybir.dt.float32
    f32r = mybir.dt.float32r
    bf16 = mybir.dt.bfloat16
    AF = mybir.ActivationFunctionType
    ALU = mybir.AluOpType
    PI4 = float(np.pi / 4.0)

    nc = bacc.Bacc("TRN2", target_bir_lowering=False, debug=False,
                   num_devices=NCORES)

    din = nc.declare_dram_parameter("din", [DIN_ROWS, NC], f32, isOutput=False)
    wpk = nc.declare_dram_parameter("wpk", [128, WCOLS], f32, isOutput=False)
    out = nc.declare_dram_parameter("out", [L, NC], f32, isOutput=True)

    HF = NC // 2  # PSUM-bank / matmul moving free-dim limit

    with tile.TileContext(nc) as tc:
        with (
            tc.tile_pool(name="const", bufs=1) as cpool,
            tc.tile_pool(name="cst", bufs=1) as cstp,
            tc.tile_pool(name="zc", bufs=1) as zcp,
            tc.tile_pool(name="io", bufs=2) as io,
            tc.tile_pool(name="wstage", bufs=2) as wstage,
            tc.tile_pool(name="abc", bufs=1) as abcp,
            tc.tile_pool(name="trig", bufs=2) as trig,
            tc.tile_pool(name="work", bufs=2) as work,
            tc.tile_pool(name="hrp", bufs=4) as hrp,
            tc.tile_pool(name="vp", bufs=2) as vp,
            tc.tile_pool(name="ps", bufs=4, space=bass.MemorySpace.PSUM) as ps,
            tc.tile_pool(name="zps", bufs=1, space=bass.MemorySpace.PSUM) as zps,
            tc.tile_pool(name="ops", bufs=1, space=bass.MemorySpace.PSUM) as opsp,
        ):
            nrmT_t = cpool.tile([3, NC], f32)
            nc.sync.dma_start(nrmT_t[:], din[L * S:L * S + 3, :])
            pc_t = cpool.tile([4, NC], f32)
            nc.sync.dma_start(pc_t[:], din[L * S + 3:L * S + 7, :])
            wabc_t = cpool.tile([3, 3 * L], f32)
            nc.gpsimd.dma_start(wabc_t[:], wpk[0:3, OFF_ABC:OFF_ABC + 3 * L])
            wsig_t = cpool.tile([128, 512], f32r)
            nc.gpsimd.dma_start(wsig_t[:], wpk[:, OFF_SIG:OFF_SIG + 512].bitcast(f32r))
            cb_t = cpool.tile([128, 8], f32)
            nc.sync.dma_start(cb_t[:], wpk[:, OFF_CB:OFF_CB + 8])

            # hemisphere-mask dots in [l, n] layout: full fp32 (sign-exact)
            a_all = cpool.tile([128, NC], f32)
            b_all = cpool.tile([128, NC], f32)
            thr_all = cpool.tile([128, NC], f32)   # TINY - c
            for hf in range(2):
                fs = hf * HF
                for wi, dst in ((0, a_all), (1, b_all), (2, thr_all)):
                    pab = ps.tile([128, HF], f32, tag="ph")
                    nc.tensor.matmul(pab[:], wabc_t[:, wi * L:(wi + 1) * L],
                                     nrmT_t[:, fs:fs + HF], start=True, stop=True)
                    if wi < 2:
                        nc.vector.tensor_copy(dst[:, fs:fs + HF], pab[:])
                    else:
                        nc.vector.tensor_scalar(dst[:, fs:fs + HF], pab[:],
                                                -1.0, TINY, ALU.mult, ALU.add)

            # persistent theta-block tiles; pc rows written once
            pc_r = cpool.tile([4, NC], f32r)
            nc.gpsimd.tensor_copy(pc_r[:], pc_t[:])
            csts = []
            for k, (lo, m) in enumerate(SUBS):
                pair = []
                for par in range(2):
                    t = cstp.tile([16 * m + 4, NC], f32r, tag=f"cst{k}_{par}")
                    nc.gpsimd.dma_start(t[16 * m:16 * m + 4, :].bitcast(f32),
                                        pc_r[:].bitcast(f32))
                    pair.append(t)
                csts.append(pair)

            z_tiles = [zcp.tile([128, NC], f32, tag=f"z{C}", name=f"z{C}")
                       for C in range(CHUNKS)]

            DQ = [nc.sync, nc.sync, nc.gpsimd]
            ABCQ = [nc.sync, nc.sync, nc.sync]

            for C in range(CHUNKS):
                par = C % 2
                # block theta loaded pre-duplicated from DRAM; one Sin each
                r_bs = []
                for k, (lo, m) in enumerate(SUBS):
                    r_b = io.tile([16 * m, NC], f32, tag=f"rb{k}", name=f"rb{k}")
                    src = din[C * 128 + 8 * lo:C * 128 + 8 * (lo + m), :]
                    DQ[k].dma_start(r_b[:],
                                    src.unsqueeze(0).broadcast_to((2, 8 * m, NC)))
                    r_bs.append(r_b)
                wcst_t = wstage.tile([KROWS, LPC * 128], f32r, tag="wcst")
                nc.sync.dma_start(wcst_t[:],
                                  wpk[0:KROWS, C * LPC * 128:(C + 1) * LPC * 128].bitcast(f32r))
                r_m = io.tile([128, NC], f32, tag="rm")
                nc.sync.dma_start(r_m[:], din[C * 128:(C + 1) * 128, :])
                for k, (lo, m) in enumerate(SUBS):
                    nc.scalar.activation(csts[k][par][0:16 * m, :], r_bs[k][:],
                                         AF.Sin,
                                         bias=cb_t[0:16 * m, 5 + (k == 2):6 + (k == 2)],
                                         scale=PI4)

                a_C = abcp.tile([128, NC], f32, tag="aC")
                b_C = abcp.tile([128, NC], f32, tag="bC")
                t_C = abcp.tile([128, NC], f32, tag="tC")
                for (src, dst), q in zip(((a_all, a_C), (b_all, b_C),
                                          (thr_all, t_C)), ABCQ):
                    dup = src[C * LPC:(C + 1) * LPC, :].unsqueeze(1)
                    q.dma_start(dst[:], dup.broadcast_to((LPC, 8, NC)))

                # mask-path trig in (l,s) layout, full fp32
                ct_m = trig.tile([128, NC], f32, tag="ct")
                st_m = trig.tile([128, NC], f32, tag="st")
                nc.scalar.activation(ct_m[:], r_m[:], AF.Sin,
                                     bias=cb_t[:, 0:1], scale=PI4)
                nc.scalar.activation(st_m[:], r_m[:], AF.Sin,
                                     bias=cb_t[:, 1:2], scale=PI4)
                # products on DVE; the combine rides a CCE accumulate-DMA
                q1 = work.tile([128, NC], f32, tag="q1", bufs=1)
                q2 = work.tile([128, NC], f32, tag="q2", bufs=1)
                nc.vector.tensor_tensor(q1[:], ct_m[:], a_C[:], ALU.mult)
                nc.vector.tensor_tensor(q2[:], st_m[:], b_C[:], ALU.mult)
                nc.gpsimd.dma_start(q1[:], q2[:], accum_op=ALU.add)

                # hemisphere mask: compare the CCE-accumulated dot vs thr
                msk = work.tile([128, NC], f32, tag="msk")
                nc.vector.tensor_tensor(msk[:], q1[:], t_C[:], ALU.is_gt)

                for hf in range(2):
                    fs = hf * HF
                    zt0 = zps.tile([64, HF], f32, tag="zt0")
                    zt1 = zps.tile([64, HF], f32, tag="zt1")
                    zts = (zt0, zt1)
                    for j16 in range(LPC):
                        k = min(j16 // 7, 2)
                        kv = 16 * SUBS[k][1] + 4
                        ph = ps.tile([128, HF], f32, tag="ph")
                        nc.tensor.matmul(
                            ph[:],
                            wcst_t[0:kv, j16 * 128:(j16 + 1) * 128],
                            csts[k][par][0:kv, fs:fs + HF],
                            start=True, stop=True)
                        hr = hrp.tile([128, HF], f32r, tag="hr")
                        if j16 % 2 == 0 or (j16 == 1 and hf == 0):
                            nc.scalar.activation(hr[:], ph[:], AF.Relu,
                                                 bias=cb_t[:, 3:4])
                        else:
                            nc.vector.tensor_scalar(hr[:], ph[:], 0.0, 0.0,
                                                    ALU.max, ALU.bypass)
                        g, p8 = j16 // 8, j16 % 8
                        nc.tensor.matmul(zts[g][:, :],
                                         wsig_t[:, p8 * 64:(p8 + 1) * 64],
                                         hr[:], start=(p8 == 0), stop=(p8 == 7))
                    # masked pre-sigmoid: (z + 30) * msk; sigmoid bias is b2-30
                    for g in range(2):
                        nc.vector.scalar_tensor_tensor(
                            z_tiles[C][64 * g:64 * g + 64, fs:fs + HF],
                            zts[g][:], 30.0,
                            msk[64 * g:64 * g + 64, fs:fs + HF],
                            ALU.add, ALU.mult)

            # tail: all sigmoids back-to-back (single act-table swap).
            # The bias tile is produced only after the last chunk's zsel, so
            # the scheduler cannot interleave tail sigmoids (and their act-
            # table swaps) into the main Sin/Relu stream.
            wsum_t = cpool.tile([128, CHUNKS * L], f32r)
            nc.sync.dma_start(wsum_t[:], wpk[:, OFF_SUM:OFF_SUM + CHUNKS * L].bitcast(f32r))
            sgb = cpool.tile([128, 1], f32)
            nc.vector.scalar_tensor_tensor(sgb[:], z_tiles[CHUNKS - 1][:, 0:1],
                                           0.0, cb_t[:, 2:3],
                                           ALU.mult, ALU.add)
            out_ps = opsp.tile([128, NC], f32)
            for C in range(CHUNKS):
                vis = vp.tile([128, NC], f32r, tag="vis")
                nc.scalar.activation(vis[:], z_tiles[C][:], AF.Sigmoid,
                                     bias=sgb[:])
                for hf in range(2):
                    fs = hf * HF
                    nc.tensor.matmul(out_ps[:, fs:fs + HF],
                                     wsum_t[:, C * L:(C + 1) * L],
                                     vis[:, fs:fs + HF],
                                     start=(C == 0), stop=(C == CHUNKS - 1))
            out_sb = cpool.tile([128, NC], f32)
            for hf in range(2):
                fs = hf * HF
                nc.vector.tensor_copy(out_sb[:, fs:fs + HF], out_ps[:, fs:fs + HF])
                nc.sync.dma_start(out[:, fs:fs + HF], out_sb[:, fs:fs + HF])

    nc.compile()
    return nc


def _host_constants(points, normals, root_rot, lgtSGLobes, lgtSGLambdas,
                    W1, b1, W2, b2):
    f8 = np.float64
    lob = lgtSGLobes.astype(f8)
    l = lob / (np.linalg.norm(lob, axis=-1, keepdims=True) + TINY)
    z = np.zeros_like(l)
    z[:, 2] = 1.0
    U = np.cross(z, l)
    U = U / (np.linalg.norm(U, axis=-1, keepdims=True) + TINY)
    V = np.cross(l, U)
    V = V / (np.linalg.norm(V, axis=-1, keepdims=True) + TINY)
    sharp = lgtSGLambdas[:, 0].astype(f8)
    r_phi = np.minimum(np.arccos(1.0 - 1.0 / sharp), np.pi / 3.0)
    sp, cp = np.sin(r_phi), np.cos(r_phi)

    Wd = root_rot.astype(f8) @ W1[3:].astype(f8)          # [3,H]
    A = sp[:, None] * (U @ Wd)                             # [L,H]
    B = sp[:, None] * (V @ Wd)
    C = cp[:, None] * (l @ Wd)
    W1p = W1[:3].astype(f8)                                # [3,H]
    b1f = b1.astype(f8)
    w2 = W2[:, 0].astype(f8)
    w_l = np.exp(sharp * (cp - 1.0))
    scale_l = w_l / (S * w_l + TINY)
    spU = sp[:, None] * U
    spV = sp[:, None] * V
    cpl = cp[:, None] * l

    # wcst: [KROWS, L*128]; col = l*128 + s*16 + h.  Sub-chunk layout {7,7,2}
    # within each 16-lobe chunk; per-lobe rows in its cst tile:
    # ct: 8*jj+s -> -A, st: 8*m+8*jj+s -> -B, pc: 16*m..16*m+4 -> W1p,b1-C.
    wcstZ = np.zeros((KROWS, L, 128), f8)
    wcstV = wcstZ.reshape(KROWS, L, 8, H)
    for ll in range(L):
        pos = ll % LPC
        k = min(pos // 7, 2)
        jj = pos - 7 * k
        m = 7 if k < 2 else 2
        for s in range(8):
            wcstV[8 * jj + s, ll, s, :] = -A[ll]
            wcstV[8 * m + 8 * jj + s, ll, s, :] = -B[ll]
        for d in range(3):
            wcstV[16 * m + d, ll, :, :] = W1p[d]
        wcstV[16 * m + 3, ll, :, :] = (b1f - C[ll])[None, :]

    # wabc: [3, 3*L]; per-lobe columns (no s duplication)
    wabc = np.concatenate([spU.T, spV.T, cpl.T], axis=1)

    # wsig: [128, 8*64]; for in-group position p: cols p*64 + l''*8 + s' =
    # w2[h]*delta(s,s')*delta(l'',p)
    wsig = np.zeros((8, H, 8, 8, 8), f8)
    for p in range(8):
        for s in range(8):
            wsig[s, :, p, p, s] = w2
    # wsum: per-chunk [128, L] blocks; block cc maps chunk-local lobe lp to
    # global output column cc*16+lp (zero elsewhere).
    wsum = np.zeros((LPC, 8, CHUNKS, L), f8)
    for cc in range(CHUNKS):
        for lp in range(LPC):
            wsum[lp, :, cc, cc * LPC + lp] = scale_l[cc * LPC + lp]

    cbias = np.zeros((128, 8), f8)
    s_of_p = np.arange(128) % 8
    # ACT Sin LUT domain is [-pi, pi]; input is r*pi/4 + bias with r in [0,1),
    # so shift each s-row by a full period where needed to stay in range.
    cos_bias = s_of_p * (np.pi / 4.0) + np.pi / 2.0 - 2.0 * np.pi * (s_of_p >= 2)
    sin_bias = s_of_p * (np.pi / 4.0) - 2.0 * np.pi * (s_of_p >= 4)
    cbias[:, 0] = cos_bias
    cbias[:, 1] = sin_bias
    cbias[:, 2] = float(b2[0]) - 30.0                     # tail sigmoid bias
    cbias[:, 3] = 0.0                                     # relu bias
    # sub-chunk tile layouts: col5 for m=7 ([ct56|st56]), col6 for m=2
    p = np.arange(128)
    cbias[:, 5] = np.where(p < 56, cos_bias, np.where(p < 112, sin_bias, 0.0))
    cbias[:, 6] = np.where(p < 16, cos_bias, np.where(p < 32, sin_bias, 0.0))

    f32 = np.float32
    wpk = np.zeros((128, WCOLS), f32)
    wpk[0:KROWS, 0:OFF_ABC] = wcstZ.reshape(KROWS, L * 128)
    wpk[0:3, OFF_ABC:OFF_SIG] = wabc
    wpk[:, OFF_SIG:OFF_SUM] = wsig.reshape(128, 512)
    wpk[:, OFF_SUM:OFF_CB] = wsum.reshape(128, CHUNKS * L)
    wpk[:, OFF_CB:WCOLS] = cbias
    return np.ascontiguousarray(wpk)


def _make_in_maps(inputs):
    wpk = _host_constants(inputs["points"], inputs["normals"],
                          inputs["root_rot"], inputs["lgtSGLobes"],
                          inputs["lgtSGLambdas"], inputs["W1"],
                          inputs["b1"], inputs["W2"], inputs["b2"])
    f32 = np.float32
    r_t = np.asarray(inputs["r_theta_random"], f32).transpose(1, 2, 0).reshape(L * S, N)
    pT = np.asarray(inputs["points"], f32).T
    nT = np.asarray(inputs["normals"], f32).T
    ones = np.ones((1, N), f32)
    din_full = np.concatenate([r_t, nT, pT, ones], axis=0)  # [1031, N]
    in_maps = []
    for c in range(NCORES):
        sl = slice(c * NC, (c + 1) * NC)
        in_maps.append({"din": np.ascontiguousarray(din_full[:, sl]),
                        "wpk": wpk})
    return in_maps


def kernel(points, normals, root_rot, lgtSGLobes, lgtSGLambdas,
           r_theta_random, W1, b1, W2, b2):
    global _PROG
    from concourse.bass_utils import run_bass_kernel_spmd

    if _PROG is None:
        _PROG = _build_program()
    nc = _PROG

    in_maps = _make_in_maps(dict(
        points=points, normals=normals, root_rot=root_rot,
        lgtSGLobes=lgtSGLobes, lgtSGLambdas=lgtSGLambdas,
        r_theta_random=r_theta_random, W1=W1, b1=b1, W2=W2, b2=b2))

    res = run_bass_kernel_spmd(nc, in_maps, list(range(NCORES)))

    f32 = np.float32
    out_full = np.empty((N, L), f32)
    for c in range(NCORES):
        out_full[c * NC:(c + 1) * NC, :] = res.results[c]["out"].T
    return out_full


# revision 19
# speedup vs baseline: 1.0487x; 1.0487x over previous
"""Trainium2 Bass kernel for the SG-visibility sampling network.

Math notes (exploited structure):
  - U,V are orthogonal to the unit lobe axis l, so dot(sample_dir, l) == cos(r_phi)
    exactly (up to fp eps).  Hence the SG weight w = exp(sharp*(cos_phi-1)) is a
    per-lobe constant and sum_s(vis*w)/(sum_s w + TINY) = scale_l * sum_s vis with
    scale_l = w/(S*w + TINY), precomputed on host.
  - pre-activation of the hidden layer decomposes as
        pre_h[n,l,s,h] = P_n[h] - C_l[h] - ct[n,l,s]*A_l[h] - st[n,l,s]*B_l[h]
    with P_n = p_n @ W1[:3] + b1,  A_l = sp_l*(U_l@Wd),  B_l = sp_l*(V_l@Wd),
    C_l = cp_l*(l_l@Wd),  Wd = root_rot @ W1[3:].
  - hemisphere mask: cos_term = ct*a_nl + st*b_nl + c_nl with
    a = normals@(sp*U)_l, b = normals@(sp*V)_l, c = normals@(cp*l)_l.

Device schedule highlights:
  - 3 packed I/O args (din = rt|normals|points, wpk = all weights, out).
  - theta block tiles are persistent (pc rows written once at setup); raw
    theta rows are loaded pre-duplicated straight from DRAM with a stride-0
    broadcast AP (1 DMA per sub-chunk instead of 2 SBUF copies).
  - stationary/moving matmul operands use f32->f32r bitcasts (no copies).
  - hemisphere mask full-width: products on DVE, the q1+q2 combine rides a
    SWDGE accumulate-DMA (CCE add), compare is one DVE is_gt vs thr=TINY-c.
  - per-half z accumulates in two [64,HF] PSUM groups; (z+30)*msk is stored
    to SBUF per chunk and ALL sigmoids run in a tail pass with bias (b2-30),
    gated behind the last chunk, so the ACT engine's function table swaps
    Sin<->Sigmoid exactly once for the whole kernel instead of ~44 times.
  - DMA issue is spread across the sync/gpsimd/scalar queues; relus split
    ACT/DVE to balance the three compute engines at ~78% each.
"""

import numpy as np

N, L, S, H = 8192, 128, 8, 16
NCORES = 8
NC = N // NCORES          # rays per core
LPC = 16                  # lobes per chunk
CHUNKS = L // LPC
TINY = 1e-6
KROWS = 116               # {7,7,2} stationary rows
SUBS = ((0, 7), (7, 7), (14, 2))

OFF_ABC = L * 128                 # wcst cols [0, 16384)
OFF_SIG = OFF_ABC + 3 * L         # wabc cols [16384, 16768)
OFF_SUM = OFF_SIG + 512           # wsig cols [16768, 17280)
OFF_CB = OFF_SUM + CHUNKS * L     # wsum cols [17280, 18304)
WCOLS = OFF_CB + 8                # cb cols [18304, 18312)
DIN_ROWS = L * S + 3 + 4          # rt rows | normals.T | [points.T; 1]

_PROG = None


def _build_program():
    import concourse.bass as bass
    import concourse.bacc as bacc
    import concourse.mybir as mybir
    import concourse.tile as tile

    f32 = mybir.dt.float32
    f32r = mybir.dt.float32r
    bf16 = mybir.dt.bfloat16
    AF = mybir.ActivationFunctionType
    ALU = mybir.AluOpType
    PI4 = float(np.pi / 4.0)

    nc = bacc.Bacc("TRN2", target_bir_lowering=False, debug=False,
                   num_devices=NCORES)

    din = nc.declare_dram_parameter("din", [DIN_ROWS, NC], f32, isOutput=False)
    wpk = nc.declare_dram_parameter("wpk", [128, WCOLS], f32, isOutput=False)
    out = nc.declare_dram_parameter("out", [L, NC], f32, isOutput=True)

    HF = NC // 2  # PSUM-bank / matmul moving free-dim limit

    with tile.TileContext(nc) as tc:
        with (
            tc.tile_pool(name="const", bufs=1) as cpool,
            tc.tile_pool(name="cst", bufs=1) as cstp,
            tc.tile_pool(name="zc", bufs=1) as zcp,
            tc.tile_pool(name="io", bufs=2) as io,
            tc.tile_pool(name="wstage", bufs=2) as wstage,
            tc.tile_pool(name="abc", bufs=1) as abcp,
            tc.tile_pool(name="trig", bufs=2) as trig,
            tc.tile_pool(name="work", bufs=2) as work,
            tc.tile_pool(name="hrp", bufs=4) as hrp,
            tc.tile_pool(name="vp", bufs=2) as vp,
            tc.tile_pool(name="ps", bufs=4, space=bass.MemorySpace.PSUM) as ps,
            tc.tile_pool(name="zps", bufs=1, space=bass.MemorySpace.PSUM) as zps,
            tc.tile_pool(name="ops", bufs=1, space=bass.MemorySpace.PSUM) as opsp,
        ):
            nrmT_t = cpool.tile([3, NC], f32)
            nc.sync.dma_start(nrmT_t[:], din[L * S:L * S + 3, :])
            pc_t = cpool.tile([4, NC], f32)
            nc.sync.dma_start(pc_t[:], din[L * S + 3:L * S + 7, :])
            wabc_t = cpool.tile([3, 3 * L], f32)
            nc.gpsimd.dma_start(wabc_t[:], wpk[0:3, OFF_ABC:OFF_ABC + 3 * L])
            wsig_t = cpool.tile([128, 512], f32r)
            nc.gpsimd.dma_start(wsig_t[:], wpk[:, OFF_SIG:OFF_SIG + 512].bitcast(f32r))
            cb_t = cpool.tile([128, 8], f32)
            nc.sync.dma_start(cb_t[:], wpk[:, OFF_CB:OFF_CB + 8])

            # hemisphere-mask dots in [l, n] layout: full fp32 (sign-exact)
            a_all = cpool.tile([128, NC], f32)
            b_all = cpool.tile([128, NC], f32)
            thr_all = cpool.tile([128, NC], f32)   # TINY - c
            for hf in range(2):
                fs = hf * HF
                for wi, dst in ((0, a_all), (1, b_all), (2, thr_all)):
                    pab = ps.tile([128, HF], f32, tag="ph")
                    nc.tensor.matmul(pab[:], wabc_t[:, wi * L:(wi + 1) * L],
                                     nrmT_t[:, fs:fs + HF], start=True, stop=True)
                    if wi < 2:
                        nc.vector.tensor_copy(dst[:, fs:fs + HF], pab[:])
                    else:
                        nc.vector.tensor_scalar(dst[:, fs:fs + HF], pab[:],
                                                -1.0, TINY, ALU.mult, ALU.add)

            # persistent theta-block tiles; pc rows written once
            pc_r = cpool.tile([4, NC], f32r)
            nc.gpsimd.tensor_copy(pc_r[:], pc_t[:])
            csts = []
            for k, (lo, m) in enumerate(SUBS):
                pair = []
                for par in range(2):
                    t = cstp.tile([16 * m + 4, NC], f32r, tag=f"cst{k}_{par}")
                    nc.gpsimd.dma_start(t[16 * m:16 * m + 4, :].bitcast(f32),
                                        pc_r[:].bitcast(f32))
                    pair.append(t)
                csts.append(pair)

            z_tiles = [zcp.tile([128, NC], f32, tag=f"z{C}", name=f"z{C}")
                       for C in range(CHUNKS)]

            DQ = [nc.sync, nc.sync, nc.gpsimd]
            ABCQ = [nc.sync, nc.sync, nc.sync]

            for C in range(CHUNKS):
                par = C % 2
                # block theta loaded pre-duplicated from DRAM; one Sin each
                r_bs = []
                for k, (lo, m) in enumerate(SUBS):
                    r_b = io.tile([16 * m, NC], f32, tag=f"rb{k}", name=f"rb{k}")
                    src = din[C * 128 + 8 * lo:C * 128 + 8 * (lo + m), :]
                    DQ[k].dma_start(r_b[:],
                                    src.unsqueeze(0).broadcast_to((2, 8 * m, NC)))
                    r_bs.append(r_b)
                wcst_t = wstage.tile([KROWS, LPC * 128], f32r, tag="wcst")
                nc.sync.dma_start(wcst_t[:],
                                  wpk[0:KROWS, C * LPC * 128:(C + 1) * LPC * 128].bitcast(f32r))
                r_m = io.tile([128, NC], f32, tag="rm")
                nc.sync.dma_start(r_m[:], din[C * 128:(C + 1) * 128, :])
                for k, (lo, m) in enumerate(SUBS):
                    nc.scalar.activation(csts[k][par][0:16 * m, :], r_bs[k][:],
                                         AF.Sin,
                                         bias=cb_t[0:16 * m, 5 + (k == 2):6 + (k == 2)],
                                         scale=PI4)

                a_C = abcp.tile([128, NC], f32, tag="aC")
                b_C = abcp.tile([128, NC], f32, tag="bC")
                t_C = abcp.tile([128, NC], f32, tag="tC")
                for (src, dst), q in zip(((a_all, a_C), (b_all, b_C),
                                          (thr_all, t_C)), ABCQ):
                    dup = src[C * LPC:(C + 1) * LPC, :].unsqueeze(1)
                    q.dma_start(dst[:], dup.broadcast_to((LPC, 8, NC)))

                # mask-path trig in (l,s) layout, full fp32
                ct_m = trig.tile([128, NC], f32, tag="ct")
                st_m = trig.tile([128, NC], f32, tag="st")
                nc.scalar.activation(ct_m[:], r_m[:], AF.Sin,
                                     bias=cb_t[:, 0:1], scale=PI4)
                nc.scalar.activation(st_m[:], r_m[:], AF.Sin,
                                     bias=cb_t[:, 1:2], scale=PI4)
                # products on DVE; the combine rides a CCE accumulate-DMA
                q1 = work.tile([128, NC], f32, tag="q1", bufs=1)
                q2 = work.tile([128, NC], f32, tag="q2", bufs=1)
                nc.vector.tensor_tensor(q1[:], ct_m[:], a_C[:], ALU.mult)
                nc.vector.tensor_tensor(q2[:], st_m[:], b_C[:], ALU.mult)
                nc.gpsimd.dma_start(q1[:], q2[:], accum_op=ALU.add)

                # hemisphere mask: compare the CCE-accumulated dot vs thr
                msk = work.tile([128, NC], f32, tag="msk")
                nc.vector.tensor_tensor(msk[:], q1[:], t_C[:], ALU.is_gt)

                for hf in range(2):
                    fs = hf * HF
                    zt0 = zps.tile([64, HF], f32, tag="zt0")
                    zt1 = zps.tile([64, HF], f32, tag="zt1")
                    zts = (zt0, zt1)
                    for j16 in range(LPC):
                        k = min(j16 // 7, 2)
                        kv = 16 * SUBS[k][1] + 4
                        ph = ps.tile([128, HF], f32, tag="ph")
                        nc.tensor.matmul(
                            ph[:],
                            wcst_t[0:kv, j16 * 128:(j16 + 1) * 128],
                            csts[k][par][0:kv, fs:fs + HF],
                            start=True, stop=True)
                        hr = hrp.tile([128, HF], f32r, tag="hr")
                        if j16 % 2 == 0 or (j16 == 1 and hf == 0):
                            nc.scalar.activation(hr[:], ph[:], AF.Relu,
                                                 bias=cb_t[:, 3:4])
                        else:
                            nc.vector.tensor_scalar(hr[:], ph[:], 0.0, 0.0,
                                                    ALU.max, ALU.bypass)
                        g, p8 = j16 // 8, j16 % 8
                        nc.tensor.matmul(zts[g][:, :],
                                         wsig_t[:, p8 * 64:(p8 + 1) * 64],
                                         hr[:], start=(p8 == 0), stop=(p8 == 7))
                    # masked pre-sigmoid: (z + 30) * msk; sigmoid bias is b2-30
                    for g in range(2):
                        nc.vector.scalar_tensor_tensor(
                            z_tiles[C][64 * g:64 * g + 64, fs:fs + HF],
                            zts[g][:], 30.0,
                            msk[64 * g:64 * g + 64, fs:fs + HF],
                            ALU.add, ALU.mult)

            # tail: all sigmoids back-to-back (single act-table swap).
            # The bias tile is produced only after the last chunk's zsel, so
            # the scheduler cannot interleave tail sigmoids (and their act-
            # table swaps) into the main Sin/Relu stream.
            wsum_t = cpool.tile([128, CHUNKS * L], f32r)
            nc.sync.dma_start(wsum_t[:], wpk[:, OFF_SUM:OFF_SUM + CHUNKS * L].bitcast(f32r))
            sgb = cpool.tile([128, 1], f32)
            nc.vector.scalar_tensor_tensor(sgb[:], z_tiles[CHUNKS - 1][:, 0:1],
                                           0.0, cb_t[:, 2:3],
                                           ALU.mult, ALU.add)
            out_ps = opsp.tile([128, NC], f32)
            for C in range(CHUNKS):
                vis = vp.tile([128, NC], f32r, tag="vis")
                nc.scalar.activation(vis[:], z_tiles[C][:], AF.Sigmoid,
                                     bias=sgb[:])
                for hf in range(2):
                    fs = hf * HF
                    nc.tensor.matmul(out_ps[:, fs:fs + HF],
                                     wsum_t[:, C * L:(C + 1) * L],
                                     vis[:, fs:fs + HF],
                                     start=(C == 0), stop=(C == CHUNKS - 1))
            out_sb = cpool.tile([128, NC], f32)
            for hf in range(2):
                fs = hf * HF
                nc.vector.tensor_copy(out_sb[:, fs:fs + HF], out_ps[:, fs:fs + HF])
                nc.sync.dma_start(out[:, fs:fs + HF], out_sb[:, fs:fs + HF])

    nc.compile()
    return nc


def _host_constants(points, normals, root_rot, lgtSGLobes, lgtSGLambdas,
                    W1, b1, W2, b2):
    f8 = np.float64
    lob = lgtSGLobes.astype(f8)
    l = lob / (np.linalg.norm(lob, axis=-1, keepdims=True) + TINY)
    z = np.zeros_like(l)
    z[:, 2] = 1.0
    U = np.cross(z, l)
    U = U / (np.linalg.norm(U, axis=-1, keepdims=True) + TINY)
    V = np.cross(l, U)
    V = V / (np.linalg.norm(V, axis=-1, keepdims=True) + TINY)
    sharp = lgtSGLambdas[:, 0].astype(f8)
    r_phi = np.minimum(np.arccos(1.0 - 1.0 / sharp), np.pi / 3.0)
    sp, cp = np.sin(r_phi), np.cos(r_phi)

    Wd = root_rot.astype(f8) @ W1[3:].astype(f8)          # [3,H]
    A = sp[:, None] * (U @ Wd)                             # [L,H]
    B = sp[:, None] * (V @ Wd)
    C = cp[:, None] * (l @ Wd)
    W1p = W1[:3].astype(f8)                                # [3,H]
    b1f = b1.astype(f8)
    w2 = W2[:, 0].astype(f8)
    w_l = np.exp(sharp * (cp - 1.0))
    scale_l = w_l / (S * w_l + TINY)
    spU = sp[:, None] * U
    spV = sp[:, None] * V
    cpl = cp[:, None] * l

    # wcst: [KROWS, L*128]; col = l*128 + s*16 + h.  Sub-chunk layout {7,7,2}
    # within each 16-lobe chunk; per-lobe rows in its cst tile:
    # ct: 8*jj+s -> -A, st: 8*m+8*jj+s -> -B, pc: 16*m..16*m+4 -> W1p,b1-C.
    wcstZ = np.zeros((KROWS, L, 128), f8)
    wcstV = wcstZ.reshape(KROWS, L, 8, H)
    for ll in range(L):
        pos = ll % LPC
        k = min(pos // 7, 2)
        jj = pos - 7 * k
        m = 7 if k < 2 else 2
        for s in range(8):
            wcstV[8 * jj + s, ll, s, :] = -A[ll]
            wcstV[8 * m + 8 * jj + s, ll, s, :] = -B[ll]
        for d in range(3):
            wcstV[16 * m + d, ll, :, :] = W1p[d]
        wcstV[16 * m + 3, ll, :, :] = (b1f - C[ll])[None, :]

    # wabc: [3, 3*L]; per-lobe columns (no s duplication)
    wabc = np.concatenate([spU.T, spV.T, cpl.T], axis=1)

    # wsig: [128, 8*64]; for in-group position p: cols p*64 + l''*8 + s' =
    # w2[h]*delta(s,s')*delta(l'',p)
    wsig = np.zeros((8, H, 8, 8, 8), f8)
    for p in range(8):
        for s in range(8):
            wsig[s, :, p, p, s] = w2
    # wsum: per-chunk [128, L] blocks; block cc maps chunk-local lobe lp to
    # global output column cc*16+lp (zero elsewhere).
    wsum = np.zeros((LPC, 8, CHUNKS, L), f8)
    for cc in range(CHUNKS):
        for lp in range(LPC):
            wsum[lp, :, cc, cc * LPC + lp] = scale_l[cc * LPC + lp]

    cbias = np.zeros((128, 8), f8)
    s_of_p = np.arange(128) % 8
    # ACT Sin LUT domain is [-pi, pi]; input is r*pi/4 + bias with r in [0,1),
    # so shift each s-row by a full period where needed to stay in range.
    cos_bias = s_of_p * (np.pi / 4.0) + np.pi / 2.0 - 2.0 * np.pi * (s_of_p >= 2)
    sin_bias = s_of_p * (np.pi / 4.0) - 2.0 * np.pi * (s_of_p >= 4)
    cbias[:, 0] = cos_bias
    cbias[:, 1] = sin_bias
    cbias[:, 2] = float(b2[0]) - 30.0                     # tail sigmoid bias
    cbias[:, 3] = 0.0                                     # relu bias
    # sub-chunk tile layouts: col5 for m=7 ([ct56|st56]), col6 for m=2
    p = np.arange(128)
    cbias[:, 5] = np.where(p < 56, cos_bias, np.where(p < 112, sin_bias, 0.0))
    cbias[:, 6] = np.where(p < 16, cos_bias, np.where(p < 32, sin_bias, 0.0))

    f32 = np.float32
    wpk = np.zeros((128, WCOLS), f32)
    wpk[0:KROWS, 0:OFF_ABC] = wcstZ.reshape(KROWS, L * 128)
    wpk[0:3, OFF_ABC:OFF_SIG] = wabc
    wpk[:, OFF_SIG:OFF_SUM] = wsig.reshape(128, 512)
    wpk[:, OFF_SUM:OFF_CB] = wsum.reshape(128, CHUNKS * L)
    wpk[:, OFF_CB:WCOLS] = cbias
    return np.ascontiguousarray(wpk)


def _make_in_maps(inputs):
    wpk = _host_constants(inputs["points"], inputs["normals"],
                          inputs["root_rot"], inputs["lgtSGLobes"],
                          inputs["lgtSGLambdas"], inputs["W1"],
                          inputs["b1"], inputs["W2"], inputs["b2"])
    f32 = np.float32
    r_t = np.asarray(inputs["r_theta_random"], f32).transpose(1, 2, 0).reshape(L * S, N)
    pT = np.asarray(inputs["points"], f32).T
    nT = np.asarray(inputs["normals"], f32).T
    ones = np.ones((1, N), f32)
    din_full = np.concatenate([r_t, nT, pT, ones], axis=0)  # [1031, N]
    in_maps = []
    for c in range(NCORES):
        sl = slice(c * NC, (c + 1) * NC)
        in_maps.append({"din": np.ascontiguousarray(din_full[:, sl]),
                        "wpk": wpk})
    return in_maps


def kernel(points, normals, root_rot, lgtSGLobes, lgtSGLambdas,
           r_theta_random, W1, b1, W2, b2):
    global _PROG
    from concourse.bass_utils import run_bass_kernel_spmd

    if _PROG is None:
        _PROG = _build_program()
    nc = _PROG

    in_maps = _make_in_maps(dict(
        points=points, normals=normals, root_rot=root_rot,
        lgtSGLobes=lgtSGLobes, lgtSGLambdas=lgtSGLambdas,
        r_theta_random=r_theta_random, W1=W1, b1=b1, W2=W2, b2=b2))

    res = run_bass_kernel_spmd(nc, in_maps, list(range(NCORES)))

    f32 = np.float32
    out_full = np.empty((N, L), f32)
    for c in range(NCORES):
        out_full[c * NC:(c + 1) * NC, :] = res.results[c]["out"].T
    return out_full
